# revision 29
# baseline (speedup 1.0000x reference)
"""Trainium2 Bass kernel for nn_MAB_72911364817388 (dense transformer block).

Reference computation (per batch element b):
    q = Q @ Wq + bq ; k = K @ Wk + bk ; v = K @ Wv + bv        (1024x512 @ 512x512)
    scores = einsum("qhd,khd->hqk", qh, kh) / sqrt(512)
    scores = where(mask==0, -1e4, scores); attn = softmax(scores, axis=k)
    oh = qh + attn @ vh ; O = LN0(oh) ; O = O + relu(O @ Wo + bo) ; O = LN1(O)

Strategy: pure data-parallel over batch B=8 -> one batch element per core.

v2 design notes (vs the fp32r baseline):
- Host pre-transposes Q and (compacted) K to d-major [512, n] and converts
  them plus all weights to bf16; all matmuls run bf16 (fp32 PSUM accum).
- No on-chip transposes at all; bias columns and the mask-bias column are
  pre-transposed on host too.
- Attention packs head pairs: scores use row-tiled (K=64) matmul pairs at
  partitions 0/64; attn@v uses col-tiled (M=64) matmul pairs; the softmax
  denominator is produced by an extra col-tiled matmul pair with an
  all-ones stationary [128,64] (so it lands pre-replicated across the
  64 partitions of each head) and divided out after a single reciprocal.
- exp() runs on big [128, 2x512] PSUM tiles to amortize the ~352-cycle
  ACTIVATE overhead; masked keys get a -100 bias column (exp -> 0 in bf16).
- Softmax is computed unnormalized (scores are ~N(0, 0.12), no max pass).
- LayerNorm stats via all-ones bf16 matmuls (d is the partition axis);
  normalize is (x - mu) * rstd in bf16 split across Pool and DVE.
"""

import numpy as np
import ml_dtypes

import concourse.bass as bass
import concourse.mybir as mybir
import concourse.tile as tile
from concourse import bacc, bass_utils

# Problem shapes (hardcoded per contract).
B = 8
NQ = NK = 1024
D = 512  # DQ = DK = DV
H = 8
HD = 64
P = 128
EPS = 1e-5
N_CORES = 8

DO = D // P   # 4  d-major partition groups
NO = NQ // P  # 8  key-major partition groups max
QC = NQ // 512  # 2 query free-dim chunks of 512

F32 = mybir.dt.float32
BF16 = mybir.dt.bfloat16
NPBF16 = ml_dtypes.bfloat16

AF = mybir.ActivationFunctionType
OP = mybir.AluOpType


def build_program(repeat: int = 1, apply_g0b0: bool = True,
                  apply_g1b1: bool = True, nkb: int = NO,
                  variant: str = "full"):
    nc = bacc.Bacc("TRN2", target_bir_lowering=False, debug=False,
                   num_devices=N_CORES)

    NKC = nkb * P
    # host-prepared, per-core inputs
    QTd = nc.dram_tensor("QT", [D, NQ], BF16, kind="ExternalInput").ap()
    KTd = nc.dram_tensor("KT", [D, NKC], BF16, kind="ExternalInput").ap()
    # WALL: [Wk | Wv | Wq | Wo] stacked; SMF: [bqT|bkT|boT|mb] columns
    # (+ [g0T|b0T|g1T|b1T] when applicable); SMB: [bvR ; wo1] rows.
    nsm = 3 * DO + nkb + (2 * DO if apply_g0b0 else 0) \
        + (2 * DO if apply_g1b1 else 0)
    WALLd = nc.dram_tensor("WALL", [4 * D, D], BF16,
                           kind="ExternalInput").ap()
    SMFd = nc.dram_tensor("SMF", [P, nsm], F32, kind="ExternalInput").ap()
    SMBd = nc.dram_tensor("SMB", [1, 2 * D], BF16,
                          kind="ExternalInput").ap()
    OTd = nc.dram_tensor("OT", [D, NQ], F32, kind="ExternalOutput").ap()

    with tile.TileContext(nc) as tc:
        def body():
            _build_body(nc, tc, QTd, KTd, WALLd, SMFd, SMBd, OTd,
                        nkb, apply_g0b0, apply_g1b1, variant)

        if repeat == 1:
            body()
        else:
            with tc.For_i(0, repeat, 1,
                          hint_engines=(mybir.EngineType.PE,
                                        mybir.EngineType.Activation,
                                        mybir.EngineType.DVE,
                                        mybir.EngineType.SP,
                                        mybir.EngineType.Pool)):
                body()

    nc.compile()
    return nc


def _build_body(nc, tc, QTd, KTd, WALLd, SMFd, SMBd, OTd,
                nkb, apply_g0b0, apply_g1b1, variant="full"):
    do_dma = variant in ("full", "dma")
    do_compute = variant in ("full", "compute")
    NKC = nkb * P
    SCALE = 1.0 / np.sqrt(np.float32(D))
    kchunks = []
    off = 0
    while off < NKC:
        w = min(512, NKC - off)
        kchunks.append((off, w))
        off += w
    qchunks = [(qc * 512, 512) for qc in range(QC)]

    import contextlib
    ctx = contextlib.ExitStack()
    with ctx:
        consts = ctx.enter_context(tc.tile_pool(name="consts", bufs=1))
        bigs = ctx.enter_context(tc.tile_pool(name="bigs", bufs=1))

        # ---------- DMAs (batched; spread over both HWDGE rings) ----------
        nsm = 3 * DO + nkb + (2 * DO if apply_g0b0 else 0) \
            + (2 * DO if apply_g1b1 else 0)
        Wall = consts.tile([P, 4 * DO, D], BF16)
        KTb = bigs.tile([P, DO, NKC], BF16, tag="ktraw")
        QTb = bigs.tile([P, DO, NQ], BF16, tag="qtraw")
        SMF = consts.tile([P, nsm], F32)
        SMB = consts.tile([1, 2 * D], BF16)
        if do_dma:
            nc.sync.dma_start(
                out=Wall, in_=WALLd.rearrange("(w p) n -> p w n", p=P))
            nc.scalar.dma_start(
                out=KTb, in_=KTd.rearrange("(o p) n -> p o n", p=P))
            nc.scalar.dma_start(
                out=QTb, in_=QTd.rearrange("(o p) n -> p o n", p=P))
            nc.sync.dma_start(out=SMF, in_=SMFd)
            nc.sync.dma_start(out=SMB, in_=SMBd)
        else:
            nc.vector.memset(Wall, 0.01)
            nc.vector.memset(KTb, 0.01)
            nc.vector.memset(QTb, 0.01)
            nc.vector.memset(SMF, 0.01)
            nc.vector.memset(SMB, 0.01)
        Wk_t = Wall[:, 0:DO, :]
        Wv_t = Wall[:, DO:2 * DO, :]
        Wq_t = Wall[:, 2 * DO:3 * DO, :]
        Wo_t = Wall[:, 3 * DO:4 * DO, :]
        bqT = SMF[:, 0:DO]
        bkT = SMF[:, DO:2 * DO]
        boT = SMF[:, 2 * DO:3 * DO]
        mb = SMF[:, 3 * DO:3 * DO + nkb]
        off_g = 3 * DO + nkb
        if apply_g0b0:
            g0T = SMF[:, off_g:off_g + DO]
            b0T = SMF[:, off_g + DO:off_g + 2 * DO]
            off_g += 2 * DO
        else:
            g0T = b0T = None
        if apply_g1b1:
            g1T = SMF[:, off_g:off_g + DO]
            b1T = SMF[:, off_g + DO:off_g + 2 * DO]
        else:
            g1T = b1T = None
        bvR = SMB[:, 0:D]
        wo1 = SMB[:, D:2 * D]

        # ---------- constants ----------
        ones_bp = consts.tile([P, P], BF16)
        nc.vector.memset(ones_bp, 1.0)
        epsT = consts.tile([P, 1], F32)
        nc.vector.memset(epsT, EPS)

        # ---------- activations (SBUF working set) ----------
        # kT0: zero-padded per-head d-major keys [P, H, NKC]: head h's 64
        # dv-rows live at partitions (h%2)*64, the other 64 rows are ZERO,
        # so scores run as plain K=128 matmuls against the full qTb rows.
        kT0 = bigs.tile([P, H, NKC], BF16, tag="ktpad")
        qTb = bigs.tile([P, DO, NQ], BF16, tag="qtproj")
        # vA: per key-block n-major values, augmented per head with a ones
        # column: [v_h (64) | 1] -> attn@v also yields the softmax
        # denominator in psum row 64.
        vA = bigs.tile([P, nkb, H * (HD + 1)], BF16, tag="vproj")
        ZT = bigs.tile([P, DO, NQ], BF16, tag="zt")
        XT = bigs.tile([P, DO, NQ], BF16, tag="xt")
        Z2 = bigs.tile([P, DO, NQ], BF16, tag="z2")
        OTt = bigs.tile([P, DO, NQ], F32, tag="ot")
        # one-time inits (values survive across repeat-loop iterations, but
        # cheap enough to leave in-body)
        for hh in range(2):
            nc.vector.memset(kT0[(1 - hh) * HD:(2 - hh) * HD, hh::2, :], 0.0)
        nc.vector.memset(
            vA.rearrange("p k (h e) -> p k h e", e=HD + 1)[:, :, :, HD], 1.0)

        if not do_compute:
            od = OTd.rearrange("(o p) q -> p o q", p=P)
            for do_ in range(DO):
                for qc in range(QC):
                    nc.sync.dma_start(
                        out=od[:, do_, qc * 512:(qc + 1) * 512],
                        in_=OTt[:, do_, qc * 512:(qc + 1) * 512])
            return

        # ---------- projections (all bf16) ----------
        with tc.tile_pool(name="ps_proj", bufs=6, space="PSUM") as ps_proj:
            # kT[dv, nk] = Wk^T @ K^T, scattered into the zero-padded kT0
            for do in range(DO):
                for off, w in kchunks:
                    ps = ps_proj.tile([P, 512], F32, tag="projps")
                    for ko in range(DO):
                        nc.tensor.matmul(
                            ps[:, :w],
                            lhsT=Wk_t[:, ko, do * P:(do + 1) * P],
                            rhs=KTb[:, ko, off:off + w],
                            start=(ko == 0), stop=(ko == DO - 1))
                    for hh in range(2):
                        h = do * 2 + hh
                        r0 = hh * HD
                        nc.scalar.activation(
                            kT0[r0:r0 + HD, h, off:off + w],
                            ps[r0:r0 + HD, :w], AF.Identity,
                            bias=bkT[r0:r0 + HD, do:do + 1])
            # v[nk, dv] = K @ Wv + 1 (x) bv   (n-major, augmented)
            vAv = vA.rearrange("p k (h e) -> p k h e", e=HD + 1)
            for no in range(nkb):
                ps = ps_proj.tile([P, 512], F32, tag="projps")
                for ko in range(DO):
                    nc.tensor.matmul(
                        ps,
                        lhsT=KTb[:, ko, no * P:(no + 1) * P],
                        rhs=Wv_t[:, ko, :],
                        start=(ko == 0), stop=False)
                nc.tensor.matmul(ps, lhsT=ones_bp[:1, :], rhs=bvR,
                                 start=False, stop=True)
                nc.scalar.copy(out=vAv[:, no, :, :HD],
                               in_=ps.rearrange("p (h e) -> p h e", e=HD))
            # qT[dv, nq] = Wq^T @ Q^T   (d-major)
            for do in range(DO):
                for off, w in qchunks:
                    ps = ps_proj.tile([P, 512], F32, tag="projps")
                    for ko in range(DO):
                        nc.tensor.matmul(
                            ps[:, :w],
                            lhsT=Wq_t[:, ko, do * P:(do + 1) * P],
                            rhs=QTb[:, ko, off:off + w],
                            start=(ko == 0), stop=(ko == DO - 1))
                    nc.scalar.activation(
                        qTb[:, do, off:off + w], ps[:, :w], AF.Identity,
                        bias=bqT[:, do:do + 1])

        # ---------- attention (head pairs hp = 0..3) ----------
        # PSUM: sc pool 2x[P,2,512] (4 banks, also hosts the 1/den
        # broadcast tile) + U pool 4x[65,512] (4 banks) = 8.
        with tc.tile_pool(name="ps_sc", bufs=2, space="PSUM") as ps_sc, \
             tc.tile_pool(name="ps_u", bufs=4, space="PSUM") as ps_u, \
             tc.tile_pool(name="pt_pool", bufs=4) as pt_pool, \
             tc.tile_pool(name="att_sb", bufs=8) as att_sb:
            for hp in range(DO):
                # U[hh][qc]: rows 0:64 = attn@v, row 64 = denominator
                Us = [[ps_u.tile([HD + 1, 512], F32, tag="u",
                                 name=f"U{hp}_{hh}_{qc}")
                       for qc in range(QC)] for hh in range(2)]

                def consume(kb, pts):
                    """U matmuls for key-block kb (one step behind the
                    scores/exp of kb+1 to keep ACT saturated)."""
                    for hh in range(2):
                        h = hp * 2 + hh
                        for qc in range(QC):
                            nc.tensor.matmul(
                                Us[hh][qc],
                                lhsT=vA[:, kb,
                                        h * (HD + 1):(h + 1) * (HD + 1)],
                                rhs=pts[hh][:, qc, :],
                                start=(kb == 0), stop=(kb == nkb - 1))

                prev = None
                for kb in range(nkb):
                    pts = []
                    for hh in range(2):
                        h = hp * 2 + hh
                        sc = ps_sc.tile([P, 2, 512], F32, tag="sc")
                        for qc in range(QC):
                            nc.tensor.matmul(
                                sc[:, qc, :],
                                lhsT=kT0[:, h, kb * P:(kb + 1) * P],
                                rhs=qTb[:, hp, qc * 512:(qc + 1) * 512],
                                start=True, stop=True)
                        pt = pt_pool.tile([P, 2, 512], BF16, tag="pt")
                        nc.scalar.activation(pt, sc, AF.Exp,
                                             bias=mb[:, kb:kb + 1],
                                             scale=SCALE)
                        pts.append(pt)
                    if prev is not None:
                        consume(kb - 1, prev)
                    prev = pts
                consume(nkb - 1, prev)
                # divide by the denominator (row 64 of U) + residual
                recs = [[att_sb.tile([1, 512], BF16, tag="rec",
                                     name=f"rec{hp}_{hh}_{qc}")
                         for qc in range(QC)] for hh in range(2)]
                for hh in range(2):
                    for qc in range(QC):
                        with nc.allow_low_precision("1/den in bf16"):
                            nc.vector.reciprocal(out=recs[hh][qc],
                                                 in_=Us[hh][qc][HD:HD + 1, :])
                B2 = ps_sc.tile([P, 2, 512], F32, tag="sc", name=f"B{hp}")
                for hh in range(2):
                    for qc in range(QC):
                        nc.tensor.matmul(
                            B2[hh * HD:(hh + 1) * HD, qc, :],
                            lhsT=ones_bp[:1, :HD], rhs=recs[hh][qc],
                            start=True, stop=True)
                Bs = att_sb.tile([P, 2, 512], BF16, tag="bs")
                nc.vector.tensor_copy(out=Bs, in_=B2)
                for qc in range(QC):
                    for hh in range(2):
                        nc.vector.tensor_mul(
                            out=ZT[hh * HD:(hh + 1) * HD, hp,
                                   qc * 512:(qc + 1) * 512],
                            in0=Us[hh][qc][:HD, :],
                            in1=Bs[hh * HD:(hh + 1) * HD, qc, :])
                    zs = ZT[:, hp, qc * 512:(qc + 1) * 512]
                    nc.gpsimd.tensor_add(
                        out=zs, in0=zs,
                        in1=qTb[:, hp, qc * 512:(qc + 1) * 512])

        # ---------- layernorm stats helper (d on partitions) ----------
        # Generator: yields (qc, s1 psum, mu_b bf16, rstd bf16), all
        # replicated across partitions.
        def layernorm(srcT, ps_pool, sq_pool, st_pool, pool_sq):
            for qc in range(QC):
                s1 = ps_pool.tile([P, 512], F32, tag="s1")
                s2 = ps_pool.tile([P, 512], F32, tag="s2")
                for ko in range(DO):
                    nc.tensor.matmul(
                        s1, lhsT=ones_bp,
                        rhs=srcT[:, ko, qc * 512:(qc + 1) * 512],
                        start=(ko == 0), stop=(ko == DO - 1))
                for ko in range(DO):
                    sq = sq_pool.tile([P, 512], BF16, tag="sq")
                    eng = nc.gpsimd if pool_sq else nc.vector
                    eng.tensor_mul(
                        out=sq, in0=srcT[:, ko, qc * 512:(qc + 1) * 512],
                        in1=srcT[:, ko, qc * 512:(qc + 1) * 512])
                    nc.tensor.matmul(s2, lhsT=ones_bp, rhs=sq,
                                     start=(ko == 0), stop=(ko == DO - 1))
                mu_b = st_pool.tile([P, 512], BF16, tag="mu_b")
                nc.vector.tensor_scalar_mul(out=mu_b, in0=s1,
                                            scalar1=1.0 / D)
                mu2 = st_pool.tile([P, 512], BF16, tag="mu2")
                nc.vector.tensor_mul(out=mu2, in0=mu_b, in1=mu_b)
                var = st_pool.tile([P, 512], F32, tag="var")
                nc.vector.scalar_tensor_tensor(
                    out=var, in0=s2, scalar=1.0 / D, in1=mu2,
                    op0=OP.mult, op1=OP.subtract)
                sd = st_pool.tile([P, 512], F32, tag="sd")
                nc.scalar.activation(sd, var, AF.Sqrt, bias=epsT)
                rstd = st_pool.tile([P, 512], BF16, tag="rstd")
                with nc.allow_low_precision("LN rstd in bf16"):
                    nc.vector.reciprocal(out=rstd, in_=sd)
                yield qc, s1, mu_b, rstd

        # ---------- LN0 (mean-shift folded into FC) ----------
        # XT' = ZT * rstd0 (uncentered).  The missing -mu0*rstd0 term is
        # constant across features per token, so (a) FC absorbs it via a
        # rank-1 correction matmul (lhsT = colsum(Wo)), and (b) LN1's own
        # mean subtraction cancels it on the residual path.  With
        # non-identity g0/b0 the fold is invalid -> explicit centering.
        fold_ln0 = g0T is None
        negc = []
        with tc.tile_pool(name="ps_ln0", bufs=4, space="PSUM") as ps_ln0, \
             tc.tile_pool(name="sq0", bufs=4) as sq0, \
             tc.tile_pool(name="st0", bufs=2) as st0:
            for qc, s1, mu_b, rstd in layernorm(ZT, ps_ln0, sq0, st0,
                                                pool_sq=False):
                if fold_ln0:
                    ncr = st0.tile([1, 512], BF16, tag="negc",
                                   name=f"negc{qc}")
                    nc.vector.scalar_tensor_tensor(
                        out=ncr, in0=s1[:1, :], scalar=-1.0 / D,
                        in1=rstd[:1, :], op0=OP.mult, op1=OP.mult)
                    negc.append(ncr)
                    for ko in range(DO):
                        nc.vector.tensor_mul(
                            out=XT[:, ko, qc * 512:(qc + 1) * 512],
                            in0=ZT[:, ko, qc * 512:(qc + 1) * 512],
                            in1=rstd)
                else:
                    for ko in range(DO):
                        ss = ZT[:, ko, qc * 512:(qc + 1) * 512]
                        ds = XT[:, ko, qc * 512:(qc + 1) * 512]
                        tm = sq0.tile([P, 512], BF16, tag="tm")
                        nc.gpsimd.tensor_sub(out=tm, in0=ss, in1=mu_b)
                        nc.vector.tensor_mul(out=ds, in0=tm, in1=rstd)
                        nc.vector.tensor_scalar(
                            out=ds, in0=ds,
                            scalar1=g0T[:, ko:ko + 1],
                            scalar2=b0T[:, ko:ko + 1],
                            op0=OP.mult, op1=OP.add)

        # ---------- FC + relu + residual ----------
        with tc.tile_pool(name="ps_fc", bufs=6, space="PSUM") as ps_fc, \
             tc.tile_pool(name="fc_sb", bufs=3) as fc_sb:
            for do in range(DO):
                for qc in range(QC):
                    ps = ps_fc.tile([P, 512], F32, tag="fcps")
                    for ko in range(DO):
                        nc.tensor.matmul(
                            ps,
                            lhsT=Wo_t[:, ko, do * P:(do + 1) * P],
                            rhs=XT[:, ko, qc * 512:(qc + 1) * 512],
                            start=(ko == 0),
                            stop=(ko == DO - 1 and not fold_ln0))
                    if fold_ln0:
                        # rank-1 mean correction: += colsum(Wo) (x) negc
                        nc.tensor.matmul(
                            ps, lhsT=wo1[:, do * P:(do + 1) * P],
                            rhs=negc[qc], start=False, stop=True)
                    fr = fc_sb.tile([P, 512], BF16, tag="fr")
                    nc.vector.tensor_scalar(
                        out=fr, in0=ps, scalar1=boT[:, do:do + 1],
                        scalar2=0.0, op0=OP.add, op1=OP.max)
                    nc.gpsimd.tensor_add(
                        out=Z2[:, do, qc * 512:(qc + 1) * 512], in0=fr,
                        in1=XT[:, do, qc * 512:(qc + 1) * 512])

        # ---------- LN1 -> output ----------
        od = OTd.rearrange("(o p) q -> p o q", p=P)
        with tc.tile_pool(name="ps_ln1", bufs=4, space="PSUM") as ps_ln1, \
             tc.tile_pool(name="sq1", bufs=4) as sq1, \
             tc.tile_pool(name="st1", bufs=2) as st1:
            for qc, s1, mu_b, rstd in layernorm(Z2, ps_ln1, sq1, st1,
                                                pool_sq=False):
                cc = st1.tile([P, 512], BF16, tag="cc")
                nc.vector.tensor_mul(out=cc, in0=mu_b, in1=rstd)
                for ko in range(DO):
                    ss = Z2[:, ko, qc * 512:(qc + 1) * 512]
                    ds = OTt[:, ko, qc * 512:(qc + 1) * 512]
                    tm = sq1.tile([P, 512], BF16, tag="tm")
                    nc.vector.tensor_mul(out=tm, in0=ss, in1=rstd)
                    if g1T is not None:
                        nc.vector.tensor_sub(out=ds, in0=tm, in1=cc)
                        nc.vector.tensor_scalar(
                            out=ds, in0=ds,
                            scalar1=g1T[:, ko:ko + 1],
                            scalar2=b1T[:, ko:ko + 1],
                            op0=OP.mult, op1=OP.add)
                    else:
                        nc.gpsimd.tensor_sub(out=ds, in0=tm, in1=cc)
                if do_dma:
                    nc.sync.dma_start(
                        out=od[:, :, qc * 512:(qc + 1) * 512],
                        in_=OTt[:, :, qc * 512:(qc + 1) * 512])


# ------------------------------------------------------------------
# host-side entry point
# ------------------------------------------------------------------
_CACHE = {}


def _get_program(repeat, apply_g0b0, apply_g1b1, nkb=NO):
    key = (repeat, apply_g0b0, apply_g1b1, nkb)
    if key not in _CACHE:
        _CACHE[key] = build_program(repeat, apply_g0b0, apply_g1b1, nkb)
    return _CACHE[key]


def compact_keys(K_b, mask_b, nkb):
    """Move unmasked keys to the front (order-preserving) and truncate to
    nkb*128 rows.  Softmax over keys is permutation-invariant and fully
    masked keys contribute exactly zero, so this is output-preserving as
    long as all unmasked keys survive the truncation."""
    nkc = nkb * P
    order = np.argsort(mask_b == 0, kind="stable")[:nkc]
    return (np.ascontiguousarray(K_b[order]),
            np.ascontiguousarray(mask_b[order]))


def pick_nkb(attention_mask):
    counts = (np.asarray(attention_mask) != 0).sum(axis=-1)
    return max(1, min(NO, int(-(-int(counts.max()) // P))))


def make_in_maps(inputs, nkb, apply_g0b0=None, apply_g1b1=None):
    if apply_g0b0 is None:
        apply_g0b0 = not (np.all(np.asarray(inputs["g0"]) == 1.0)
                          and np.all(np.asarray(inputs["b0"]) == 0.0))
    if apply_g1b1 is None:
        apply_g1b1 = not (np.all(np.asarray(inputs["g1"]) == 1.0)
                          and np.all(np.asarray(inputs["b1"]) == 0.0))

    def colT(name):
        return np.ascontiguousarray(
            np.asarray(inputs[name], np.float32).reshape(DO, P).T)

    wall = np.concatenate([
        np.asarray(inputs["Wk"], NPBF16),
        np.asarray(inputs["Wv"], NPBF16),
        np.asarray(inputs["Wq"], NPBF16),
        np.asarray(inputs["Wo"], NPBF16)], axis=0)
    smb = np.concatenate([
        np.asarray(inputs["bv"], NPBF16),
        np.asarray(np.asarray(inputs["Wo"], np.float32).sum(axis=0),
                   NPBF16)]).reshape(1, 2 * D)
    smf_cols = [colT("bq"), colT("bk"), colT("bo")]
    shared = {"WALL": wall, "SMB": smb}
    if apply_g0b0:
        g0b0 = [colT("g0"), colT("b0")]
    else:
        g0b0 = []
    if apply_g1b1:
        g1b1 = [colT("g1"), colT("b1")]
    else:
        g1b1 = []

    Q = np.asarray(inputs["Q"], np.float32)
    K = np.asarray(inputs["K"], np.float32)
    mask = np.asarray(inputs["attention_mask"], np.int32)
    in_maps = []
    for b in range(B):
        m = dict(shared)
        m["QT"] = np.ascontiguousarray(Q[b].T).astype(NPBF16)
        Kc, mc = compact_keys(K[b], mask[b], nkb)
        m["KT"] = np.ascontiguousarray(Kc.T).astype(NPBF16)
        # (mask-1)*100: 1 -> 0, 0 -> -100 ; transposed to [P, nkb]
        mbcol = ((mc.astype(np.float32) - 1.0) * 100.0).reshape(nkb, P).T
        m["SMF"] = np.ascontiguousarray(np.concatenate(
            smf_cols + [mbcol] + g0b0 + g1b1, axis=1, dtype=np.float32))
        in_maps.append(m)
    return in_maps


def kernel(Q, K, attention_mask, Wq, bq, Wk, bk, Wv, bv, Wo, bo,
           g0, b0, g1, b1, _repeat=1):
    inputs = {
        "Q": Q, "K": K, "attention_mask": attention_mask,
        "Wq": Wq, "bq": bq, "Wk": Wk, "bk": bk, "Wv": Wv, "bv": bv,
        "Wo": Wo, "bo": bo, "g0": g0, "b0": b0, "g1": g1, "b1": b1,
    }
    apply_g0b0 = not (np.all(np.asarray(g0) == 1.0)
                      and np.all(np.asarray(b0) == 0.0))
    apply_g1b1 = not (np.all(np.asarray(g1) == 1.0)
                      and np.all(np.asarray(b1) == 0.0))
    nkb = pick_nkb(attention_mask)
    nc = _get_program(_repeat, apply_g0b0, apply_g1b1, nkb)
    in_maps = make_in_maps(inputs, nkb, apply_g0b0, apply_g1b1)

    res = bass_utils.run_bass_kernel_spmd(
        nc, in_maps, core_ids=list(range(N_CORES)), trace=False)

    out = np.empty((B, NQ, D), np.float32)
    for b in range(B):
        out[b] = res.results[b]["OT"].T
    return out


# revision 31
# speedup vs baseline: 1.1780x; 1.1780x over previous
"""Trainium2 Bass kernel for nn_MAB_72911364817388 (dense transformer block).

Reference computation (per batch element b):
    q = Q @ Wq + bq ; k = K @ Wk + bk ; v = K @ Wv + bv        (1024x512 @ 512x512)
    scores = einsum("qhd,khd->hqk", qh, kh) / sqrt(512)
    scores = where(mask==0, -1e4, scores); attn = softmax(scores, axis=k)
    oh = qh + attn @ vh ; O = LN0(oh) ; O = O + relu(O @ Wo + bo) ; O = LN1(O)

Strategy: pure data-parallel over batch B=8 -> one batch element per core.

v2 design notes (vs the fp32r baseline):
- Host pre-transposes Q and (compacted) K to d-major [512, n] and converts
  them plus all weights to bf16; all matmuls run bf16 (fp32 PSUM accum).
- No on-chip transposes at all; bias columns and the mask-bias column are
  pre-transposed on host too.
- Attention packs head pairs: scores use row-tiled (K=64) matmul pairs at
  partitions 0/64; attn@v uses col-tiled (M=64) matmul pairs; the softmax
  denominator is produced by an extra col-tiled matmul pair with an
  all-ones stationary [128,64] (so it lands pre-replicated across the
  64 partitions of each head) and divided out after a single reciprocal.
- exp() runs on big [128, 2x512] PSUM tiles to amortize the ~352-cycle
  ACTIVATE overhead; masked keys get a -100 bias column (exp -> 0 in bf16).
- Softmax is computed unnormalized (scores are ~N(0, 0.12), no max pass).
- LayerNorm stats via all-ones bf16 matmuls (d is the partition axis);
  normalize is (x - mu) * rstd in bf16 split across Pool and DVE.
"""

import numpy as np
import ml_dtypes

import concourse.bass as bass
import concourse.mybir as mybir
import concourse.tile as tile
from concourse import bacc, bass_utils

# Problem shapes (hardcoded per contract).
B = 8
NQ = NK = 1024
D = 512  # DQ = DK = DV
H = 8
HD = 64
P = 128
EPS = 1e-5
N_CORES = 8

DO = D // P   # 4  d-major partition groups
NO = NQ // P  # 8  key-major partition groups max
QC = NQ // 512  # 2 query free-dim chunks of 512

F32 = mybir.dt.float32
BF16 = mybir.dt.bfloat16
NPBF16 = ml_dtypes.bfloat16

AF = mybir.ActivationFunctionType
OP = mybir.AluOpType


def build_program(repeat: int = 1, apply_g0b0: bool = True,
                  apply_g1b1: bool = True, nkb: int = NO,
                  variant: str = "full"):
    nc = bacc.Bacc("TRN2", target_bir_lowering=False, debug=False,
                   num_devices=N_CORES)

    NKC = nkb * P
    # host-prepared, per-core inputs
    QTd = nc.dram_tensor("QT", [D, NQ], BF16, kind="ExternalInput").ap()
    KTd = nc.dram_tensor("KT", [D, NKC], BF16, kind="ExternalInput").ap()
    # WALL: [Wk | Wv | Wq | Wo] stacked; SMF: [bqT|bkT|boT|mb] columns
    # (+ [g0T|b0T|g1T|b1T] when applicable); SMB: [bvR ; wo1] rows.
    nsm = 3 * DO + nkb + (2 * DO if apply_g0b0 else 0) \
        + (2 * DO if apply_g1b1 else 0)
    WALLd = nc.dram_tensor("WALL", [3 * D, D], BF16,
                           kind="ExternalInput").ap()
    WOd = nc.dram_tensor("WO", [D, D], BF16, kind="ExternalInput").ap()
    SMFd = nc.dram_tensor("SMF", [P, nsm], F32, kind="ExternalInput").ap()
    SMBd = nc.dram_tensor("SMB", [1, 2 * D], BF16,
                          kind="ExternalInput").ap()
    OTd = nc.dram_tensor("OT", [D, NQ], F32, kind="ExternalOutput").ap()

    with tile.TileContext(nc) as tc:
        def body():
            _build_body(nc, tc, QTd, KTd, WALLd, WOd, SMFd, SMBd, OTd,
                        nkb, apply_g0b0, apply_g1b1, variant)

        if repeat == 1:
            body()
        elif repeat == 2:
            body()
            body()
        else:
            # unrolled x2: halves the all-engine loop barriers and lets one
            # body's tail drain overlap the next body's DMAs/projections
            with tc.For_i(0, repeat // 2, 1,
                          hint_engines=(mybir.EngineType.PE,
                                        mybir.EngineType.Activation,
                                        mybir.EngineType.DVE,
                                        mybir.EngineType.SP,
                                        mybir.EngineType.Pool)):
                body()
                body()
            if repeat % 2:
                body()

    nc.compile()
    return nc


def _build_body(nc, tc, QTd, KTd, WALLd, WOd, SMFd, SMBd, OTd,
                nkb, apply_g0b0, apply_g1b1, variant="full"):
    do_dma = variant in ("full", "dma")
    do_compute = variant in ("full", "compute")
    NKC = nkb * P
    SCALE = 1.0 / np.sqrt(np.float32(D))
    kchunks = []
    off = 0
    while off < NKC:
        w = min(512, NKC - off)
        kchunks.append((off, w))
        off += w
    qchunks = [(qc * 512, 512) for qc in range(QC)]

    import contextlib
    ctx = contextlib.ExitStack()
    with ctx:
        consts = ctx.enter_context(tc.tile_pool(name="consts", bufs=1))
        bigs = ctx.enter_context(tc.tile_pool(name="bigs", bufs=1))

        # ---------- DMAs (batched; spread over both HWDGE rings) ----------
        nsm = 3 * DO + nkb + (2 * DO if apply_g0b0 else 0) \
            + (2 * DO if apply_g1b1 else 0)
        Wall = consts.tile([P, 3 * DO, D], BF16)
        WoT = consts.tile([P, DO, D], BF16)
        KTb = bigs.tile([P, DO, NKC], BF16, tag="ktraw")
        QTb = bigs.tile([P, DO, NQ], BF16, tag="qtraw")
        SMF = consts.tile([P, nsm], F32)
        SMB = consts.tile([1, 2 * D], BF16)
        if do_dma:
            nc.sync.dma_start(
                out=Wall, in_=WALLd.rearrange("(w p) n -> p w n", p=P))
            nc.scalar.dma_start(
                out=KTb, in_=KTd.rearrange("(o p) n -> p o n", p=P))
            nc.scalar.dma_start(
                out=QTb, in_=QTd.rearrange("(o p) n -> p o n", p=P))
            nc.sync.dma_start(out=SMF, in_=SMFd)
            nc.sync.dma_start(out=SMB, in_=SMBd)
            nc.sync.dma_start(
                out=WoT, in_=WOd.rearrange("(k p) n -> p k n", p=P))
        else:
            nc.vector.memset(Wall, 0.01)
            nc.vector.memset(KTb, 0.01)
            nc.vector.memset(QTb, 0.01)
            nc.vector.memset(SMF, 0.01)
            nc.vector.memset(SMB, 0.01)
            nc.vector.memset(WoT, 0.01)
        Wk_t = Wall[:, 0:DO, :]
        Wv_t = Wall[:, DO:2 * DO, :]
        Wq_t = Wall[:, 2 * DO:3 * DO, :]
        Wo_t = WoT
        bqT = SMF[:, 0:DO]
        bkT = SMF[:, DO:2 * DO]
        boT = SMF[:, 2 * DO:3 * DO]
        mb = SMF[:, 3 * DO:3 * DO + nkb]
        off_g = 3 * DO + nkb
        if apply_g0b0:
            g0T = SMF[:, off_g:off_g + DO]
            b0T = SMF[:, off_g + DO:off_g + 2 * DO]
            off_g += 2 * DO
        else:
            g0T = b0T = None
        if apply_g1b1:
            g1T = SMF[:, off_g:off_g + DO]
            b1T = SMF[:, off_g + DO:off_g + 2 * DO]
        else:
            g1T = b1T = None
        bvR = SMB[:, 0:D]
        wo1 = SMB[:, D:2 * D]

        # ---------- constants ----------
        ones_bp = consts.tile([P, P], BF16)
        nc.vector.memset(ones_bp, 1.0)
        epsT = consts.tile([P, 1], F32)
        nc.vector.memset(epsT, EPS)

        # ---------- activations (SBUF working set) ----------
        # kT0: zero-padded per-head d-major keys [P, H, NKC]: head h's 64
        # dv-rows live at partitions (h%2)*64, the other 64 rows are ZERO,
        # so scores run as plain K=128 matmuls against the full qTb rows.
        kT0 = bigs.tile([P, H, NKC], BF16, tag="ktpad")
        qTb = bigs.tile([P, DO, NQ], BF16, tag="qtproj")
        # vA: per key-block n-major values, augmented per head with a ones
        # column: [v_h (64) | 1] -> attn@v also yields the softmax
        # denominator in psum row 64.
        vA = bigs.tile([P, nkb, H * (HD + 1)], BF16, tag="vproj")
        ZT = bigs.tile([P, DO, NQ], BF16, tag="zt")
        XT = bigs.tile([P, DO, NQ], BF16, tag="xt")
        Z2 = bigs.tile([P, DO, NQ], BF16, tag="z2")
        OTt = bigs.tile([P, DO, NQ], F32, tag="ot")
        # one-time inits (values survive across repeat-loop iterations, but
        # cheap enough to leave in-body)
        for hh in range(2):
            nc.vector.memset(kT0[(1 - hh) * HD:(2 - hh) * HD, hh::2, :], 0.0)
        nc.vector.memset(
            vA.rearrange("p k (h e) -> p k h e", e=HD + 1)[:, :, :, HD], 1.0)

        if not do_compute:
            od = OTd.rearrange("(o p) q -> p o q", p=P)
            for do_ in range(DO):
                for qc in range(QC):
                    nc.sync.dma_start(
                        out=od[:, do_, qc * 512:(qc + 1) * 512],
                        in_=OTt[:, do_, qc * 512:(qc + 1) * 512])
            return

        # ---------- projections (all bf16) ----------
        with tc.tile_pool(name="ps_proj", bufs=6, space="PSUM") as ps_proj:
            # kT[dv, nk] = Wk^T @ K^T, scattered into the zero-padded kT0
            for do in range(DO):
                for off, w in kchunks:
                    ps = ps_proj.tile([P, 512], F32, tag="projps")
                    for ko in range(DO):
                        nc.tensor.matmul(
                            ps[:, :w],
                            lhsT=Wk_t[:, ko, do * P:(do + 1) * P],
                            rhs=KTb[:, ko, off:off + w],
                            start=(ko == 0), stop=(ko == DO - 1))
                    for hh in range(2):
                        h = do * 2 + hh
                        r0 = hh * HD
                        nc.scalar.activation(
                            kT0[r0:r0 + HD, h, off:off + w],
                            ps[r0:r0 + HD, :w], AF.Identity,
                            bias=bkT[r0:r0 + HD, do:do + 1])
            # v[nk, dv] = K @ Wv + 1 (x) bv   (n-major, augmented)
            vAv = vA.rearrange("p k (h e) -> p k h e", e=HD + 1)
            for no in range(nkb):
                ps = ps_proj.tile([P, 512], F32, tag="projps")
                for ko in range(DO):
                    nc.tensor.matmul(
                        ps,
                        lhsT=KTb[:, ko, no * P:(no + 1) * P],
                        rhs=Wv_t[:, ko, :],
                        start=(ko == 0), stop=False)
                nc.tensor.matmul(ps, lhsT=ones_bp[:1, :], rhs=bvR,
                                 start=False, stop=True)
                nc.scalar.copy(out=vAv[:, no, :, :HD],
                               in_=ps.rearrange("p (h e) -> p h e", e=HD))
            # qT[dv, nq] = Wq^T @ Q^T   (d-major)
            for do in range(DO):
                for off, w in qchunks:
                    ps = ps_proj.tile([P, 512], F32, tag="projps")
                    for ko in range(DO):
                        nc.tensor.matmul(
                            ps[:, :w],
                            lhsT=Wq_t[:, ko, do * P:(do + 1) * P],
                            rhs=QTb[:, ko, off:off + w],
                            start=(ko == 0), stop=(ko == DO - 1))
                    nc.scalar.activation(
                        qTb[:, do, off:off + w], ps[:, :w], AF.Identity,
                        bias=bqT[:, do:do + 1])

        # ---------- attention (head pairs hp = 0..3) ----------
        # PSUM: sc pool 2x[P,2,512] (4 banks, also hosts the 1/den
        # broadcast tile) + U pool 4x[65,512] (4 banks) = 8.
        with tc.tile_pool(name="ps_sc", bufs=2, space="PSUM") as ps_sc, \
             tc.tile_pool(name="ps_u", bufs=4, space="PSUM") as ps_u, \
             tc.tile_pool(name="pt_pool", bufs=4) as pt_pool, \
             tc.tile_pool(name="att_sb", bufs=8) as att_sb:
            for hp in range(DO):
                # U[hh][qc]: rows 0:64 = attn@v, row 64 = denominator
                Us = [[ps_u.tile([HD + 1, 512], F32, tag="u",
                                 name=f"U{hp}_{hh}_{qc}")
                       for qc in range(QC)] for hh in range(2)]

                def consume(kb, pts):
                    """U matmuls for key-block kb (one step behind the
                    scores/exp of kb+1 to keep ACT saturated)."""
                    for hh in range(2):
                        h = hp * 2 + hh
                        for qc in range(QC):
                            nc.tensor.matmul(
                                Us[hh][qc],
                                lhsT=vA[:, kb,
                                        h * (HD + 1):(h + 1) * (HD + 1)],
                                rhs=pts[hh][:, qc, :],
                                start=(kb == 0), stop=(kb == nkb - 1))

                prev = None
                for kb in range(nkb):
                    pts = []
                    for hh in range(2):
                        h = hp * 2 + hh
                        sc = ps_sc.tile([P, 2, 512], F32, tag="sc")
                        for qc in range(QC):
                            nc.tensor.matmul(
                                sc[:, qc, :],
                                lhsT=kT0[:, h, kb * P:(kb + 1) * P],
                                rhs=qTb[:, hp, qc * 512:(qc + 1) * 512],
                                start=True, stop=True)
                        pt = pt_pool.tile([P, 2, 512], BF16, tag="pt")
                        nc.scalar.activation(pt, sc, AF.Exp,
                                             bias=mb[:, kb:kb + 1],
                                             scale=SCALE)
                        pts.append(pt)
                    if prev is not None:
                        consume(kb - 1, prev)
                    prev = pts
                consume(nkb - 1, prev)
                # divide by the denominator (row 64 of U) + residual
                recs = [[att_sb.tile([1, 512], BF16, tag="rec",
                                     name=f"rec{hp}_{hh}_{qc}")
                         for qc in range(QC)] for hh in range(2)]
                for hh in range(2):
                    for qc in range(QC):
                        with nc.allow_low_precision("1/den in bf16"):
                            nc.vector.reciprocal(out=recs[hh][qc],
                                                 in_=Us[hh][qc][HD:HD + 1, :])
                B2 = ps_sc.tile([P, 2, 512], F32, tag="sc", name=f"B{hp}")
                for hh in range(2):
                    for qc in range(QC):
                        nc.tensor.matmul(
                            B2[hh * HD:(hh + 1) * HD, qc, :],
                            lhsT=ones_bp[:1, :HD], rhs=recs[hh][qc],
                            start=True, stop=True)
                Bs = att_sb.tile([P, 2, 512], BF16, tag="bs")
                nc.vector.tensor_copy(out=Bs, in_=B2)
                for qc in range(QC):
                    for hh in range(2):
                        nc.vector.tensor_mul(
                            out=ZT[hh * HD:(hh + 1) * HD, hp,
                                   qc * 512:(qc + 1) * 512],
                            in0=Us[hh][qc][:HD, :],
                            in1=Bs[hh * HD:(hh + 1) * HD, qc, :])
                    zs = ZT[:, hp, qc * 512:(qc + 1) * 512]
                    nc.gpsimd.tensor_add(
                        out=zs, in0=zs,
                        in1=qTb[:, hp, qc * 512:(qc + 1) * 512])

        # ---------- layernorm stats helper (d on partitions) ----------
        # Generator: yields (qc, s1 psum, mu_b bf16, rstd bf16), all
        # replicated across partitions.
        def layernorm(srcT, ps_pool, sq_pool, st_pool, pool_sq):
            for qc in range(QC):
                s1 = ps_pool.tile([P, 512], F32, tag="s1")
                s2 = ps_pool.tile([P, 512], F32, tag="s2")
                for ko in range(DO):
                    nc.tensor.matmul(
                        s1, lhsT=ones_bp,
                        rhs=srcT[:, ko, qc * 512:(qc + 1) * 512],
                        start=(ko == 0), stop=(ko == DO - 1))
                for ko in range(DO):
                    sq = sq_pool.tile([P, 512], BF16, tag="sq")
                    eng = nc.gpsimd if pool_sq else nc.vector
                    eng.tensor_mul(
                        out=sq, in0=srcT[:, ko, qc * 512:(qc + 1) * 512],
                        in1=srcT[:, ko, qc * 512:(qc + 1) * 512])
                    nc.tensor.matmul(s2, lhsT=ones_bp, rhs=sq,
                                     start=(ko == 0), stop=(ko == DO - 1))
                mu_b = st_pool.tile([P, 512], BF16, tag="mu_b")
                nc.vector.tensor_scalar_mul(out=mu_b, in0=s1,
                                            scalar1=1.0 / D)
                mu2 = st_pool.tile([P, 512], BF16, tag="mu2")
                nc.vector.tensor_mul(out=mu2, in0=mu_b, in1=mu_b)
                var = st_pool.tile([P, 512], F32, tag="var")
                nc.vector.scalar_tensor_tensor(
                    out=var, in0=s2, scalar=1.0 / D, in1=mu2,
                    op0=OP.mult, op1=OP.subtract)
                sd = st_pool.tile([P, 512], F32, tag="sd")
                nc.scalar.activation(sd, var, AF.Sqrt, bias=epsT)
                rstd = st_pool.tile([P, 512], BF16, tag="rstd")
                with nc.allow_low_precision("LN rstd in bf16"):
                    nc.vector.reciprocal(out=rstd, in_=sd)
                yield qc, s1, mu_b, rstd

        # prefetch the Sqrt activation table off the LN critical chain
        with tc.tile_pool(name="warm_sb", bufs=1) as warm_sb:
            warm = warm_sb.tile([1, 1], F32)
            nc.scalar.activation(warm, epsT[:1, :], AF.Sqrt)

        # ---------- LN0 (mean-shift folded into FC) ----------
        # XT' = ZT * rstd0 (uncentered).  The missing -mu0*rstd0 term is
        # constant across features per token, so (a) FC absorbs it via a
        # rank-1 correction matmul (lhsT = colsum(Wo)), and (b) LN1's own
        # mean subtraction cancels it on the residual path.  With
        # non-identity g0/b0 the fold is invalid -> explicit centering.
        fold_ln0 = g0T is None
        negc = []
        with tc.tile_pool(name="ps_ln0", bufs=4, space="PSUM") as ps_ln0, \
             tc.tile_pool(name="sq0", bufs=4) as sq0, \
             tc.tile_pool(name="st0", bufs=2) as st0:
            for qc, s1, mu_b, rstd in layernorm(ZT, ps_ln0, sq0, st0,
                                                pool_sq=False):
                if fold_ln0:
                    ncr = st0.tile([1, 512], BF16, tag="negc",
                                   name=f"negc{qc}")
                    nc.vector.scalar_tensor_tensor(
                        out=ncr, in0=s1[:1, :], scalar=-1.0 / D,
                        in1=rstd[:1, :], op0=OP.mult, op1=OP.mult)
                    negc.append(ncr)
                    for ko in range(DO):
                        nc.vector.tensor_mul(
                            out=XT[:, ko, qc * 512:(qc + 1) * 512],
                            in0=ZT[:, ko, qc * 512:(qc + 1) * 512],
                            in1=rstd)
                else:
                    for ko in range(DO):
                        ss = ZT[:, ko, qc * 512:(qc + 1) * 512]
                        ds = XT[:, ko, qc * 512:(qc + 1) * 512]
                        tm = sq0.tile([P, 512], BF16, tag="tm")
                        nc.gpsimd.tensor_sub(out=tm, in0=ss, in1=mu_b)
                        nc.vector.tensor_mul(out=ds, in0=tm, in1=rstd)
                        nc.vector.tensor_scalar(
                            out=ds, in0=ds,
                            scalar1=g0T[:, ko:ko + 1],
                            scalar2=b0T[:, ko:ko + 1],
                            op0=OP.mult, op1=OP.add)

        # ---------- FC + relu + residual ----------
        with tc.tile_pool(name="ps_fc", bufs=6, space="PSUM") as ps_fc, \
             tc.tile_pool(name="fc_sb", bufs=3) as fc_sb:
            for do in range(DO):
                for qc in range(QC):
                    ps = ps_fc.tile([P, 512], F32, tag="fcps")
                    for ko in range(DO):
                        nc.tensor.matmul(
                            ps,
                            lhsT=Wo_t[:, ko, do * P:(do + 1) * P],
                            rhs=XT[:, ko, qc * 512:(qc + 1) * 512],
                            start=(ko == 0),
                            stop=(ko == DO - 1 and not fold_ln0))
                    if fold_ln0:
                        # rank-1 mean correction: += colsum(Wo) (x) negc
                        nc.tensor.matmul(
                            ps, lhsT=wo1[:, do * P:(do + 1) * P],
                            rhs=negc[qc], start=False, stop=True)
                    fr = fc_sb.tile([P, 512], BF16, tag="fr")
                    nc.vector.tensor_scalar(
                        out=fr, in0=ps, scalar1=boT[:, do:do + 1],
                        scalar2=0.0, op0=OP.add, op1=OP.max)
                    nc.gpsimd.tensor_add(
                        out=Z2[:, do, qc * 512:(qc + 1) * 512], in0=fr,
                        in1=XT[:, do, qc * 512:(qc + 1) * 512])

        # ---------- LN1 -> output ----------
        od = OTd.rearrange("(o p) q -> p o q", p=P)
        with tc.tile_pool(name="ps_ln1", bufs=4, space="PSUM") as ps_ln1, \
             tc.tile_pool(name="sq1", bufs=4) as sq1, \
             tc.tile_pool(name="st1", bufs=2) as st1:
            for qc, s1, mu_b, rstd in layernorm(Z2, ps_ln1, sq1, st1,
                                                pool_sq=False):
                cc = st1.tile([P, 512], BF16, tag="cc")
                nc.vector.tensor_mul(out=cc, in0=mu_b, in1=rstd)
                for ko in range(DO):
                    ss = Z2[:, ko, qc * 512:(qc + 1) * 512]
                    ds = OTt[:, ko, qc * 512:(qc + 1) * 512]
                    tm = sq1.tile([P, 512], BF16, tag="tm")
                    nc.vector.tensor_mul(out=tm, in0=ss, in1=rstd)
                    if g1T is not None:
                        nc.vector.tensor_sub(out=ds, in0=tm, in1=cc)
                        nc.vector.tensor_scalar(
                            out=ds, in0=ds,
                            scalar1=g1T[:, ko:ko + 1],
                            scalar2=b1T[:, ko:ko + 1],
                            op0=OP.mult, op1=OP.add)
                    else:
                        nc.gpsimd.tensor_sub(out=ds, in0=tm, in1=cc)
                if do_dma:
                    nc.sync.dma_start(
                        out=od[:, :, qc * 512:(qc + 1) * 512],
                        in_=OTt[:, :, qc * 512:(qc + 1) * 512])


# ------------------------------------------------------------------
# host-side entry point
# ------------------------------------------------------------------
_CACHE = {}


def _get_program(repeat, apply_g0b0, apply_g1b1, nkb=NO):
    key = (repeat, apply_g0b0, apply_g1b1, nkb)
    if key not in _CACHE:
        _CACHE[key] = build_program(repeat, apply_g0b0, apply_g1b1, nkb)
    return _CACHE[key]


def compact_keys(K_b, mask_b, nkb):
    """Move unmasked keys to the front (order-preserving) and truncate to
    nkb*128 rows.  Softmax over keys is permutation-invariant and fully
    masked keys contribute exactly zero, so this is output-preserving as
    long as all unmasked keys survive the truncation."""
    nkc = nkb * P
    order = np.argsort(mask_b == 0, kind="stable")[:nkc]
    return (np.ascontiguousarray(K_b[order]),
            np.ascontiguousarray(mask_b[order]))


def pick_nkb(attention_mask):
    counts = (np.asarray(attention_mask) != 0).sum(axis=-1)
    return max(1, min(NO, int(-(-int(counts.max()) // P))))


def make_in_maps(inputs, nkb, apply_g0b0=None, apply_g1b1=None):
    if apply_g0b0 is None:
        apply_g0b0 = not (np.all(np.asarray(inputs["g0"]) == 1.0)
                          and np.all(np.asarray(inputs["b0"]) == 0.0))
    if apply_g1b1 is None:
        apply_g1b1 = not (np.all(np.asarray(inputs["g1"]) == 1.0)
                          and np.all(np.asarray(inputs["b1"]) == 0.0))

    def colT(name):
        return np.ascontiguousarray(
            np.asarray(inputs[name], np.float32).reshape(DO, P).T)

    wall = np.concatenate([
        np.asarray(inputs["Wk"], NPBF16),
        np.asarray(inputs["Wv"], NPBF16),
        np.asarray(inputs["Wq"], NPBF16)], axis=0)
    smb = np.concatenate([
        np.asarray(inputs["bv"], NPBF16),
        np.asarray(np.asarray(inputs["Wo"], np.float32).sum(axis=0),
                   NPBF16)]).reshape(1, 2 * D)
    smf_cols = [colT("bq"), colT("bk"), colT("bo")]
    shared = {"WALL": wall, "SMB": smb,
              "WO": np.asarray(inputs["Wo"], NPBF16)}
    if apply_g0b0:
        g0b0 = [colT("g0"), colT("b0")]
    else:
        g0b0 = []
    if apply_g1b1:
        g1b1 = [colT("g1"), colT("b1")]
    else:
        g1b1 = []

    Q = np.asarray(inputs["Q"], np.float32)
    K = np.asarray(inputs["K"], np.float32)
    mask = np.asarray(inputs["attention_mask"], np.int32)
    in_maps = []
    for b in range(B):
        m = dict(shared)
        m["QT"] = np.ascontiguousarray(Q[b].T).astype(NPBF16)
        Kc, mc = compact_keys(K[b], mask[b], nkb)
        m["KT"] = np.ascontiguousarray(Kc.T).astype(NPBF16)
        # (mask-1)*100: 1 -> 0, 0 -> -100 ; transposed to [P, nkb]
        mbcol = ((mc.astype(np.float32) - 1.0) * 100.0).reshape(nkb, P).T
        m["SMF"] = np.ascontiguousarray(np.concatenate(
            smf_cols + [mbcol] + g0b0 + g1b1, axis=1, dtype=np.float32))
        in_maps.append(m)
    return in_maps


def kernel(Q, K, attention_mask, Wq, bq, Wk, bk, Wv, bv, Wo, bo,
           g0, b0, g1, b1, _repeat=1):
    inputs = {
        "Q": Q, "K": K, "attention_mask": attention_mask,
        "Wq": Wq, "bq": bq, "Wk": Wk, "bk": bk, "Wv": Wv, "bv": bv,
        "Wo": Wo, "bo": bo, "g0": g0, "b0": b0, "g1": g1, "b1": b1,
    }
    apply_g0b0 = not (np.all(np.asarray(g0) == 1.0)
                      and np.all(np.asarray(b0) == 0.0))
    apply_g1b1 = not (np.all(np.asarray(g1) == 1.0)
                      and np.all(np.asarray(b1) == 0.0))
    nkb = pick_nkb(attention_mask)
    nc = _get_program(_repeat, apply_g0b0, apply_g1b1, nkb)
    in_maps = make_in_maps(inputs, nkb, apply_g0b0, apply_g1b1)

    res = bass_utils.run_bass_kernel_spmd(
        nc, in_maps, core_ids=list(range(N_CORES)), trace=False)

    out = np.empty((B, NQ, D), np.float32)
    for b in range(B):
        out[b] = res.results[b]["OT"].T
    return out
